# revision 22
# baseline (speedup 1.0000x reference)
"""Trainium2 Bass kernel for nn_BigramLanguageModel (B=4, T=2048, C=512, V=32000).

Sharding: 8 cores = 4 batches x 2 parities. Core c handles batch b=c//2 and
the query tiles with global tile index g = 2j + (c%2), j=0..7 (parity-
interleaved across the two cores of a batch for causal load balance).

Uniformity trick: one SPMD program runs on all 8 cores, so nothing about the
program may depend on the parity. Each core works in a LOCAL token order
chosen by the host: local tile 2j holds global tile 2j+p, local tile 2j+1
holds global tile 2j+1-p (identity for p=0, pairwise swap for p=1). Own query
tiles are then always the EVEN local tiles — a compile-time stride — while
every position-dependent quantity (idx order, pos_emb columns, key positions
spos, query positions qpos) arrives as per-core input data. The set of the
first 8(w+1) local tiles equals the set of the first 8(w+1) global tiles for
either parity, so causal trip counts are compile-time constants too.

Per core: embedding gather + PE-transpose -> x^T; K^T/V/Q^T projections;
attention in S^T [keys, queries] layout (softmax denominators via ones-vector
matmuls, divide folded into attn^T eviction — no P transposes); lm_head over
the full vocab for its 1024 own rows; CE partials: logsumexp accumulated by
exp-with-accumulate during the lm_head sweep, target logits via host-gathered
W_tgt = Wo[:, targets] and an elementwise mul + ones-matmul reduction.

All matmuls are fp32r (TF32-like precision, full PE rate at moving dim>=256).
Host assembles the [4,2048,32000] logits and the scalar mean CE loss.
"""

import sys

sys.path.insert(0, "/opt/trn_rl_repo")

import numpy as np

import concourse.bass as bass
import concourse.bacc as bacc
import concourse.mybir as mybir
import concourse.tile as tile

F32 = mybir.dt.float32
F32R = mybir.dt.float32r
I32 = mybir.dt.int32
P = 128

B, T, C, V = 4, 2048, 512, 32000
N_CORES = 8
CCH = C // P  # 4 chunks of the channel dim
MASK_NEG = -3000.0


def build_program(T_=T, V_=V, with_bias=False, do_compile=True):
    """Build the uniform per-core SPMD program (C=512 fixed)."""
    NT = T_ // P          # local key tiles (16)
    TOWN = T_ // 2        # own query rows (1024)
    NQ = TOWN // P        # own query tiles (8)
    NW = TOWN // 512      # query windows of 512 (2)
    assert TOWN % 512 == 0 and T_ % 1024 == 0
    vch = []
    off = 0
    while off < V_:
        nv = min(512, V_ - off)
        assert nv >= 256
        vch.append((off, nv))
        off += nv
    NVC = len(vch)
    scale = float(C) ** -0.5

    nc = bacc.Bacc(None, target_bir_lowering=False)

    idx_d = nc.dram_tensor("idx", [T_], I32, kind="ExternalInput")
    temb_d = nc.dram_tensor("token_emb", [V_, C], F32, kind="ExternalInput")
    post_d = nc.dram_tensor("pos_emb_t", [C, T_], F32, kind="ExternalInput")
    wq_d = nc.dram_tensor("wq", [C, C], F32R, kind="ExternalInput")
    wk_d = nc.dram_tensor("wk", [C, C], F32R, kind="ExternalInput")
    wv_d = nc.dram_tensor("wv", [C, C], F32R, kind="ExternalInput")
    wo_d = nc.dram_tensor("wo", [C, V_], F32R, kind="ExternalInput")
    bo_d = (
        nc.dram_tensor("bo_bc", [P, V_], F32, kind="ExternalInput")
        if with_bias
        else None
    )
    wtgt_d = nc.dram_tensor("w_tgt", [C, TOWN], F32R, kind="ExternalInput")
    botgt_d = nc.dram_tensor("bo_tgt", [1, TOWN], F32, kind="ExternalInput")
    qpos_d = nc.dram_tensor("qpos", [1, TOWN], F32, kind="ExternalInput")
    spos_d = nc.dram_tensor("spos", [P, NT], F32, kind="ExternalInput")
    ident_d = nc.dram_tensor("ident", [P, P], F32, kind="ExternalInput")
    ones_d = nc.dram_tensor("ones_r", [P, 1], F32R, kind="ExternalInput")

    logits_d = nc.dram_tensor("logits", [TOWN, V_], F32, kind="ExternalOutput")
    loss_d = nc.dram_tensor("loss_part", [1, 1], F32, kind="ExternalOutput")

    Exp = mybir.ActivationFunctionType.Exp
    Copy = mybir.ActivationFunctionType.Copy
    Ln = mybir.ActivationFunctionType.Ln
    Alu = mybir.AluOpType

    with tile.TileContext(nc) as tc:
        with (
            tc.tile_pool(name="const", bufs=1) as constp,
            tc.tile_pool(name="persist", bufs=1) as persist,
        ):
            # ---------------- constants ----------------
            ident = constp.tile([P, P], F32)
            nc.sync.dma_start(out=ident[:], in_=ident_d[:])
            ones_r = constp.tile([P, 1], F32R)
            nc.sync.dma_start(out=ones_r[:], in_=ones_d[:])
            spos = constp.tile([P, NT], F32)
            nc.sync.dma_start(out=spos[:], in_=spos_d[:])
            qpos1 = constp.tile([1, TOWN], F32)
            nc.sync.dma_start(out=qpos1[:], in_=qpos_d[:])
            ones_row = constp.tile([1, P], F32)
            nc.vector.memset(ones_row[:], 1.0)
            oneone = constp.tile([1, 1], F32)
            nc.vector.memset(oneone[:], 1.0)
            qpos_bc = constp.tile([P, TOWN], F32)
            with tc.tile_pool(name="psb", bufs=2, space="PSUM") as psb:
                for w in range(NW):
                    bps = psb.tile([P, 512], F32, tag="bps", name="bps")
                    nc.tensor.matmul(
                        out=bps[:],
                        lhsT=ones_row[:],
                        rhs=qpos1[:, w * 512 : (w + 1) * 512],
                        start=True,
                        stop=True,
                    )
                    nc.vector.tensor_copy(
                        out=qpos_bc[:, w * 512 : (w + 1) * 512], in_=bps[:]
                    )
            idx_sb = constp.tile([P, NT], I32)
            nc.sync.dma_start(
                out=idx_sb[:], in_=idx_d[:].rearrange("(g r) -> r g", r=P)
            )

            attnT = {
                (i, w): persist.tile([P, 512], F32R, tag=f"attnT{i}_{w}", name=f"attnT{i}_{w}")
                for i in range(CCH)
                for w in range(NW)
            }
            acc8 = [persist.tile([P, NVC], F32, tag=f"acc{j}", name=f"acc{j}") for j in range(NQ)]

            # ---------- phases 1-3 under kvqp (K^T/V/Q^T span phases 2-3) ----------
            with tc.tile_pool(name="kvqp", bufs=1) as kvqp:
                kT = [kvqp.tile([P, T_], F32R, tag=f"kT{i}", name=f"kT{i}") for i in range(CCH)]
                vt = [kvqp.tile([P, C], F32R, tag=f"v{g}", name=f"v{g}") for g in range(NT)]
                qT = [kvqp.tile([P, TOWN], F32R, tag=f"qT{i}", name=f"qT{i}") for i in range(CCH)]
                with tc.tile_pool(name="xTp", bufs=1) as xTp:
                    xT = [xTp.tile([P, T_], F32R, tag=f"xT{i}", name=f"xT{i}") for i in range(CCH)]
                    # ----- phase 1: x^T = (gather token_emb)^T + pos^T -----
                    with (
                        tc.tile_pool(name="ph1", bufs=3) as ph1,
                        tc.tile_pool(name="posTp", bufs=1) as posTp,
                        tc.tile_pool(name="ps1", bufs=4, space="PSUM") as ps1,
                    ):
                        posT = []
                        for i in range(CCH):
                            pt = posTp.tile([P, T_], F32, tag=f"posT{i}", name=f"posT{i}")
                            nc.sync.dma_start(
                                out=pt[:], in_=post_d[i * P : (i + 1) * P, :]
                            )
                            posT.append(pt)
                        for g in range(NT):
                            xg = ph1.tile([P, C], F32, tag="xg", name="xg")
                            nc.gpsimd.indirect_dma_start(
                                out=xg[:],
                                out_offset=None,
                                in_=temb_d[:],
                                in_offset=bass.IndirectOffsetOnAxis(
                                    ap=idx_sb[:, g : g + 1], axis=0
                                ),
                            )
                            for i in range(CCH):
                                tp = ps1.tile([P, P], F32, tag="xtps", name="xtps")
                                nc.tensor.transpose(
                                    out=tp[:],
                                    in_=xg[:, i * P : (i + 1) * P],
                                    identity=ident[:],
                                )
                                nc.vector.tensor_tensor(
                                    out=xT[i][:, g * P : (g + 1) * P],
                                    in0=tp[:],
                                    in1=posT[i][:, g * P : (g + 1) * P],
                                    op=Alu.add,
                                )

                    # ----- phase 2: K^T [c,s], V [s,c], Q^T [c,q_own] -----
                    with (
                        tc.tile_pool(name="wqkv", bufs=1) as wqkv,
                        tc.tile_pool(name="ps2", bufs=2, space="PSUM") as ps2,
                    ):
                        wq_t = wqkv.tile([P, CCH, C], F32R)
                        wk_t = wqkv.tile([P, CCH, C], F32R)
                        wv_t = wqkv.tile([P, CCH, C], F32R)
                        for w_t, w_d in ((wq_t, wq_d), (wk_t, wk_d), (wv_t, wv_d)):
                            nc.sync.dma_start(
                                out=w_t[:], in_=w_d[:].rearrange("(k p) n -> p k n", p=P)
                            )
                        # emit per half (tok windows 0-1 then 2-3) so window-0
                        # attention can start while the second half is produced
                        for half in range(2):
                            w4s = range(half * (T_ // 1024), (half + 1) * (T_ // 1024))
                            for co in range(CCH):
                                for w4 in w4s:
                                    ps = ps2.tile([P, 512], F32, tag="kps", name="kps")
                                    for ci in range(CCH):
                                        nc.tensor.matmul(
                                            out=ps[:],
                                            lhsT=wk_t[:, ci, co * P : (co + 1) * P],
                                            rhs=xT[ci][:, w4 * 512 : (w4 + 1) * 512],
                                            start=(ci == 0),
                                            stop=(ci == CCH - 1),
                                        )
                                    nc.scalar.activation(
                                        out=kT[co][:, w4 * 512 : (w4 + 1) * 512],
                                        in_=ps[:],
                                        func=Copy,
                                    )
                            for g in range(half * (NT // 2), (half + 1) * (NT // 2)):
                                ps = ps2.tile([P, C], F32, tag="vps", name="vps")
                                for ci in range(CCH):
                                    nc.tensor.matmul(
                                        out=ps[:],
                                        lhsT=xT[ci][:, g * P : (g + 1) * P],
                                        rhs=wv_t[:, ci, :],
                                        start=(ci == 0),
                                        stop=(ci == CCH - 1),
                                    )
                                nc.scalar.activation(out=vt[g][:], in_=ps[:], func=Copy)
                            for co in range(CCH if half < NW else 0):
                                w = half
                                ps = ps2.tile([P, 512], F32, tag="qps", name="qps")
                                for ci in range(CCH):
                                    xv = xT[ci][:].rearrange("c (g r) -> c g r", r=P)
                                    rhs = xv[:, 2 * 4 * w : 2 * 4 * (w + 1) : 2, :]
                                    nc.tensor.matmul(
                                        out=ps[:],
                                        lhsT=wq_t[:, ci, co * P : (co + 1) * P],
                                        rhs=rhs,
                                        start=(ci == 0),
                                        stop=(ci == CCH - 1),
                                    )
                                nc.scalar.activation(
                                    out=qT[co][:, w * 512 : (w + 1) * 512],
                                    in_=ps[:],
                                    func=Copy,
                                    scale=scale,
                                )

                # ---------- phase 3: attention in S^T layout ----------
                se_w = [persist.tile([1, 512], F32, tag=f"se_w{w}", name=f"se_w{w}") for w in range(NW)]
                recip8 = persist.tile([P, NQ], F32, tag="recip8", name="recip8")
                with (
                        tc.tile_pool(name="pTp", bufs=1) as pTp,
                        tc.tile_pool(name="ph3", bufs=2) as ph3,
                        tc.tile_pool(name="ps3a", bufs=3, space="PSUM") as ps3a,
                        tc.tile_pool(name="ps3b", bufs=2, space="PSUM") as ps3b,
                        tc.tile_pool(name="ps3c", bufs=2, space="PSUM") as ps3c,
                    ):
                        for w in range(NW):
                            ns = 8 * (w + 1)
                            qsl = slice(w * 512, (w + 1) * 512)
                            se_ps = ps3c.tile([1, 512], F32, tag="se_ps", name="se_ps")
                            pT_w = []
                            for ls in range(ns):
                                sps = ps3b.tile([P, 512], F32, tag="s_ps", name="s_ps")
                                for ci in range(CCH):
                                    nc.tensor.matmul(
                                        out=sps[:],
                                        lhsT=kT[ci][:, ls * P : (ls + 1) * P],
                                        rhs=qT[ci][:, qsl],
                                        start=(ci == 0),
                                        stop=(ci == CCH - 1),
                                    )
                                pt = pTp.tile([P, 512], F32R, tag=f"pT{ls}", name=f"pT{ls}")
                                if ls >= 8 * w:
                                    mask = ph3.tile([P, 512], F32, tag="mask", name="mask")
                                    nc.vector.tensor_scalar(
                                        out=mask[:],
                                        in0=qpos_bc[:, qsl],
                                        scalar1=spos[:, ls : ls + 1],
                                        scalar2=MASK_NEG,
                                        op0=Alu.is_lt,
                                        op1=Alu.mult,
                                    )
                                    smask = ph3.tile([P, 512], F32, tag="smask", name="smask")
                                    nc.vector.tensor_tensor(
                                        out=smask[:], in0=sps[:], in1=mask[:], op=Alu.add
                                    )
                                    nc.scalar.activation(out=pt[:], in_=smask[:], func=Exp)
                                else:
                                    nc.scalar.activation(out=pt[:], in_=sps[:], func=Exp)
                                pT_w.append(pt)
                                nc.tensor.matmul(
                                    out=se_ps[:],
                                    lhsT=ones_r[:],
                                    rhs=pt[:],
                                    start=(ls == 0),
                                    stop=(ls == ns - 1),
                                )
                            nc.vector.tensor_copy(out=se_w[w][:], in_=se_ps[:])
                            # per-token 1/sumexp for this window's 4 token tiles
                            for jj in range(4):
                                j = 4 * w + jj
                                scp = ps3c.tile([P, 1], F32, tag="scp", name="scp", bufs=1)
                                nc.tensor.matmul(
                                    out=scp[:],
                                    lhsT=se_w[w][:, jj * P : (jj + 1) * P],
                                    rhs=oneone[:],
                                    start=True,
                                    stop=True,
                                )
                                nc.vector.reciprocal(out=recip8[:, j : j + 1], in_=scp[:])
                            for co in range(CCH):
                                aps = ps3a.tile([P, 512], F32, tag="attn_ps", name="attn_ps")
                                for ls in range(ns):
                                    nc.tensor.matmul(
                                        out=aps[:],
                                        lhsT=vt[ls][:, co * P : (co + 1) * P],
                                        rhs=pT_w[ls][:],
                                        start=(ls == 0),
                                        stop=(ls == ns - 1),
                                    )
                                # attn^T stays unnormalized; the softmax divide is
                                # folded into the lm_head eviction per token tile
                                nc.vector.tensor_copy(out=attnT[(co, w)][:], in_=aps[:])

            # ---------- phase 4: lm_head + CE ----------
            with (
                tc.tile_pool(name="ph4", bufs=3) as ph4,
                tc.tile_pool(name="wop", bufs=2) as wop,
                tc.tile_pool(name="ps4", bufs=6, space="PSUM") as ps4,
                tc.tile_pool(name="ps5", bufs=1, space="PSUM") as ps5,
                tc.tile_pool(name="ps6", bufs=1, space="PSUM") as ps6,
            ):
                # vocab groups of up to 4 x 512 chunks -> big contiguous DMAs
                vgroups = []
                k = 0
                while k < NVC:
                    grp = vch[k : k + 4]
                    vgroups.append(grp)
                    k += 4
                for grp in vgroups:
                    goff = grp[0][0]
                    gnv = sum(nv for _, nv in grp)
                    wo_g = []
                    for ci in range(CCH):
                        wg = wop.tile([P, 2048], F32R, tag=f"wo{ci}", name=f"wo{ci}", bufs=3)
                        nc.sync.dma_start(
                            out=wg[:, :gnv],
                            in_=wo_d[ci * P : (ci + 1) * P, goff : goff + gnv],
                        )
                        wo_g.append(wg)
                    if with_bias:
                        bo_bc = ph4.tile([P, 2048], F32, tag="bo_bc", name="bo_bc", bufs=2)
                        nc.sync.dma_start(
                            out=bo_bc[:, :gnv], in_=bo_d[:, goff : goff + gnv]
                        )
                    for j in range(NQ):
                        jw, jj = j // 4, j % 4
                        lg4 = ph4.tile([P, 2048], F32, tag="lg4", name="lg4", bufs=3)
                        for off, nv in grp:
                            loff = off - goff
                            vc = off // 512
                            ps = ps4.tile([P, nv], F32, tag="lm_ps", name="lm_ps")
                            for ci in range(CCH):
                                nc.tensor.matmul(
                                    out=ps[:],
                                    lhsT=attnT[(ci, jw)][:, jj * P : (jj + 1) * P],
                                    rhs=wo_g[ci][:, loff : loff + nv],
                                    start=(ci == 0),
                                    stop=(ci == CCH - 1),
                                )
                            nc.vector.tensor_scalar(
                                out=lg4[:, loff : loff + nv],
                                in0=ps[:],
                                scalar1=recip8[:, j : j + 1],
                                scalar2=None,
                                op0=Alu.mult,
                            )
                            if with_bias:
                                nc.vector.tensor_tensor(
                                    out=lg4[:, loff : loff + nv],
                                    in0=lg4[:, loff : loff + nv],
                                    in1=bo_bc[:, loff : loff + nv],
                                    op=Alu.add,
                                )
                                escr = ph4.tile([P, 512], F32, tag="escr", name="escr", bufs=2)
                                nc.scalar.activation(
                                    out=escr[:, :nv],
                                    in_=lg4[:, loff : loff + nv],
                                    func=Exp,
                                    accum_out=acc8[j][:, vc : vc + 1],
                                )
                            else:
                                escr = ph4.tile([P, 512], F32, tag="escr", name="escr", bufs=2)
                                nc.scalar.activation(
                                    out=escr[:, :nv],
                                    in_=ps[:],
                                    func=Exp,
                                    scale=recip8[:, j : j + 1],
                                    accum_out=acc8[j][:, vc : vc + 1],
                                )
                        nc.sync.dma_start(
                            out=logits_d[j * P : (j + 1) * P, goff : goff + gnv],
                            in_=lg4[:, :gnv],
                        )

                # ---- CE tail ----
                lse8 = ph4.tile([P, NQ], F32, tag="lse8", name="lse8", bufs=1)
                for j in range(NQ):
                    se = ph4.tile([P, 1], F32, tag="se_j", name="se_j", bufs=2)
                    nc.vector.reduce_sum(
                        out=se[:], in_=acc8[j][:], axis=mybir.AxisListType.X
                    )
                    nc.scalar.activation(out=lse8[:, j : j + 1], in_=se[:], func=Ln)
                # cross-partition sum of all NQ lse columns: one plain-f32 matmul
                loss_ps = ps5.tile([1, NQ], F32, tag="loss_ps", name="loss_ps")
                nc.tensor.matmul(
                    out=loss_ps[:],
                    lhsT=ones_r[:].bitcast(F32),
                    rhs=lse8[:],
                    start=True,
                    stop=True,
                )
                tgt_sb = ph4.tile([1, TOWN], F32, tag="tgt_sb", name="tgt_sb", bufs=1)
                for w in range(NW):
                    qsl = slice(w * 512, (w + 1) * 512)
                    tgt_ps = ps6.tile([1, 512], F32, tag="tgt_ps", name="tgt_ps")
                    for ci in range(CCH):
                        wtc = ph4.tile([P, 512], F32R, tag="wtc", name="wtc", bufs=2)
                        nc.sync.dma_start(
                            out=wtc[:],
                            in_=wtgt_d[ci * P : (ci + 1) * P, qsl],
                        )
                        tg = ph4.tile([P, 512], F32R, tag="tg", name="tg", bufs=2)
                        nc.vector.tensor_tensor(
                            out=tg[:],
                            in0=attnT[(ci, w)][:],
                            in1=wtc[:],
                            op=Alu.mult,
                        )
                        nc.tensor.matmul(
                            out=tgt_ps[:],
                            lhsT=ones_r[:],
                            rhs=tg[:],
                            start=(ci == 0),
                            stop=(ci == CCH - 1),
                        )
                    nc.vector.tensor_copy(out=tgt_sb[:, qsl], in_=tgt_ps[:])
                rec_row = ph4.tile([1, TOWN], F32, tag="rec_row", name="rec_row", bufs=1)
                for w in range(NW):
                    nc.vector.reciprocal(
                        out=rec_row[:, w * 512 : (w + 1) * 512], in_=se_w[w][:]
                    )
                nc.vector.tensor_tensor(
                    out=tgt_sb[:], in0=tgt_sb[:], in1=rec_row[:], op=Alu.mult
                )
                botgt_sb = ph4.tile([1, TOWN], F32, tag="botgt", name="botgt", bufs=1)
                nc.sync.dma_start(out=botgt_sb[:], in_=botgt_d[:])
                nc.vector.tensor_tensor(
                    out=tgt_sb[:], in0=tgt_sb[:], in1=botgt_sb[:], op=Alu.add
                )
                tgt_sum = ph4.tile([1, 1], F32, tag="tgt_sum", name="tgt_sum", bufs=1)
                nc.vector.reduce_sum(
                    out=tgt_sum[:], in_=tgt_sb[:], axis=mybir.AxisListType.X
                )
                lse_row = ph4.tile([1, NQ], F32, tag="lse_row", name="lse_row", bufs=1)
                nc.vector.tensor_copy(out=lse_row[:], in_=loss_ps[:])
                lse_sum = ph4.tile([1, 1], F32, tag="lse_sum", name="lse_sum", bufs=1)
                nc.vector.reduce_sum(
                    out=lse_sum[:], in_=lse_row[:], axis=mybir.AxisListType.X
                )
                loss_sb = ph4.tile([1, 1], F32, tag="loss_sb", name="loss_sb", bufs=1)
                nc.vector.tensor_tensor(
                    out=loss_sb[:], in0=lse_sum[:], in1=tgt_sum[:], op=Alu.subtract
                )
                nc.sync.dma_start(out=loss_d[:], in_=loss_sb[:])

    if do_compile:
        nc.compile()
    return nc


# ---------------------------------------------------------------------------
# host side
# ---------------------------------------------------------------------------

def _local_order(T_):
    """pos_of_local[p][t_local] = global position, per parity."""
    NT = T_ // P
    out = {}
    for p in (0, 1):
        gt = np.arange(NT)
        gtile = np.where(gt % 2 == 0, gt + p, gt - p)  # local tile -> global tile
        out[p] = (gtile[:, None] * P + np.arange(P)[None, :]).reshape(-1)
    return out


def make_in_maps(idx, targets, token_emb, pos_emb, Wk, Wq, Wv, Wo, bo, T_=T, V_=V):
    idx = np.asarray(idx).astype(np.int32)
    targets = np.asarray(targets).astype(np.int64)
    token_emb = np.ascontiguousarray(np.asarray(token_emb, np.float32))
    pos_emb_t = np.ascontiguousarray(np.asarray(pos_emb, np.float32).T)
    Wk = np.ascontiguousarray(np.asarray(Wk, np.float32))
    Wq = np.ascontiguousarray(np.asarray(Wq, np.float32))
    Wv = np.ascontiguousarray(np.asarray(Wv, np.float32))
    Wo = np.ascontiguousarray(np.asarray(Wo, np.float32))
    bo = np.asarray(bo, np.float32).reshape(1, V_)
    with_bias = bool(np.any(bo))

    NT = T_ // P
    TOWN = T_ // 2
    NQ = TOWN // P
    NVC = (V_ + 511) // 512
    order = _local_order(T_)
    spos_base = {
        p: (order[p].reshape(NT, P).T).astype(np.float32) for p in (0, 1)
    }  # [P, NT] global position of each local key slot
    ident = np.eye(P, dtype=np.float32)
    ones = np.ones((P, 1), np.float32)

    in_maps = []
    meta = []
    n_batches = idx.shape[0]
    for c in range(N_CORES):
        b, p = c // 2, c % 2
        pos_l = order[p]
        own_rows_local = np.concatenate(
            [np.arange(2 * j * P, (2 * j + 1) * P) for j in range(NQ)]
        )
        own_global = pos_l[own_rows_local]
        tgt_own = targets[b % n_batches][own_global]
        in_map = {
            "idx": idx[b % n_batches][pos_l],
            "token_emb": token_emb,
            "pos_emb_t": pos_emb_t[:, pos_l],
            "wq": Wq,
            "wk": Wk,
            "wv": Wv,
            "wo": Wo,
            "w_tgt": np.ascontiguousarray(Wo[:, tgt_own]),
            "bo_tgt": bo[:, tgt_own],
            "qpos": own_global[None, :].astype(np.float32),
            "spos": spos_base[p],
            "ident": ident,
            "ones_r": ones,
        }
        if with_bias:
            in_map["bo_bc"] = np.ascontiguousarray(np.broadcast_to(bo, (P, V_)))
        in_maps.append(in_map)
        meta.append((b % n_batches, own_global))
    return in_maps, meta, with_bias


_prog_cache = {}


def kernel(idx, targets, token_emb, pos_emb, Wk, Wq, Wv, Wo, bo):
    from concourse.bass_utils import run_bass_kernel_spmd

    in_maps, meta, with_bias = make_in_maps(
        idx, targets, token_emb, pos_emb, Wk, Wq, Wv, Wo, bo, T, V
    )
    key = (T, V, with_bias)
    if key not in _prog_cache:
        _prog_cache[key] = build_program(T, V, with_bias=with_bias)
    nc = _prog_cache[key]
    res = run_bass_kernel_spmd(nc, in_maps, list(range(N_CORES)))

    logits = np.empty((B, T, V), np.float32)
    loss_sum = 0.0
    for c in range(N_CORES):
        b, own_global = meta[c]
        logits[b][own_global] = res.results[c]["logits"]
        loss_sum += float(res.results[c]["loss_part"][0, 0])
    loss = np.float32(loss_sum / (B * T))
    return logits, loss


# revision 23
# speedup vs baseline: 1.0468x; 1.0468x over previous
"""Trainium2 Bass kernel for nn_BigramLanguageModel (B=4, T=2048, C=512, V=32000).

Sharding: 8 cores = 4 batches x 2 parities. Core c handles batch b=c//2 and
the query tiles with global tile index g = 2j + (c%2), j=0..7 (parity-
interleaved across the two cores of a batch for causal load balance).

Uniformity trick: one SPMD program runs on all 8 cores, so nothing about the
program may depend on the parity. Each core works in a LOCAL token order
chosen by the host: local tile 2j holds global tile 2j+p, local tile 2j+1
holds global tile 2j+1-p (identity for p=0, pairwise swap for p=1). Own query
tiles are then always the EVEN local tiles — a compile-time stride — while
every position-dependent quantity (idx order, pos_emb columns, key positions
spos, query positions qpos) arrives as per-core input data. The set of the
first 8(w+1) local tiles equals the set of the first 8(w+1) global tiles for
either parity, so causal trip counts are compile-time constants too.

Per core: embedding gather + PE-transpose -> x^T; K^T/V/Q^T projections;
attention in S^T [keys, queries] layout (softmax denominators via ones-vector
matmuls, divide folded into attn^T eviction — no P transposes); lm_head over
the full vocab for its 1024 own rows; CE partials: logsumexp accumulated by
exp-with-accumulate during the lm_head sweep, target logits via host-gathered
W_tgt = Wo[:, targets] and an elementwise mul + ones-matmul reduction.

All matmuls are fp32r (TF32-like precision, full PE rate at moving dim>=256).
Host assembles the [4,2048,32000] logits and the scalar mean CE loss.
"""

import sys

sys.path.insert(0, "/opt/trn_rl_repo")

import numpy as np

import concourse.bass as bass
import concourse.bacc as bacc
import concourse.mybir as mybir
import concourse.tile as tile

F32 = mybir.dt.float32
F32R = mybir.dt.float32r
I32 = mybir.dt.int32
P = 128

B, T, C, V = 4, 2048, 512, 32000
N_CORES = 8
CCH = C // P  # 4 chunks of the channel dim
MASK_NEG = -3000.0


def build_program(T_=T, V_=V, with_bias=False, do_compile=True):
    """Build the uniform per-core SPMD program (C=512 fixed)."""
    NT = T_ // P          # local key tiles (16)
    TOWN = T_ // 2        # own query rows (1024)
    NQ = TOWN // P        # own query tiles (8)
    NW = TOWN // 512      # query windows of 512 (2)
    assert TOWN % 512 == 0 and T_ % 1024 == 0
    vch = []
    off = 0
    while off < V_:
        nv = min(512, V_ - off)
        assert nv >= 256
        vch.append((off, nv))
        off += nv
    NVC = len(vch)
    scale = float(C) ** -0.5

    nc = bacc.Bacc(None, target_bir_lowering=False)

    idx_d = nc.dram_tensor("idx", [T_], I32, kind="ExternalInput")
    temb_d = nc.dram_tensor("token_emb", [V_, C], F32, kind="ExternalInput")
    post_d = nc.dram_tensor("pos_emb_t", [C, T_], F32, kind="ExternalInput")
    wq_d = nc.dram_tensor("wq", [C, C], F32R, kind="ExternalInput")
    wk_d = nc.dram_tensor("wk", [C, C], F32R, kind="ExternalInput")
    wv_d = nc.dram_tensor("wv", [C, C], F32R, kind="ExternalInput")
    wo_d = nc.dram_tensor("wo", [C, V_], F32R, kind="ExternalInput")
    bo_d = (
        nc.dram_tensor("bo_bc", [P, V_], F32, kind="ExternalInput")
        if with_bias
        else None
    )
    wtgt_d = nc.dram_tensor("w_tgt", [C, TOWN], F32R, kind="ExternalInput")
    botgt_d = nc.dram_tensor("bo_tgt", [1, TOWN], F32, kind="ExternalInput")
    qpos_d = nc.dram_tensor("qpos", [1, TOWN], F32, kind="ExternalInput")
    spos_d = nc.dram_tensor("spos", [P, NT], F32, kind="ExternalInput")
    ident_d = nc.dram_tensor("ident", [P, P], F32, kind="ExternalInput")
    ones_d = nc.dram_tensor("ones_r", [P, 1], F32R, kind="ExternalInput")

    logits_d = nc.dram_tensor("logits", [TOWN, V_], F32, kind="ExternalOutput")
    loss_d = nc.dram_tensor("loss_part", [1, 1], F32, kind="ExternalOutput")

    Exp = mybir.ActivationFunctionType.Exp
    Copy = mybir.ActivationFunctionType.Copy
    Ln = mybir.ActivationFunctionType.Ln
    Alu = mybir.AluOpType

    with tile.TileContext(nc) as tc:
        with (
            tc.tile_pool(name="const", bufs=1) as constp,
            tc.tile_pool(name="persist", bufs=1) as persist,
        ):
            # ---------------- constants ----------------
            ident = constp.tile([P, P], F32)
            nc.sync.dma_start(out=ident[:], in_=ident_d[:])
            ones_r = constp.tile([P, 1], F32R)
            nc.sync.dma_start(out=ones_r[:], in_=ones_d[:])
            spos = constp.tile([P, NT], F32)
            nc.sync.dma_start(out=spos[:], in_=spos_d[:])
            qpos1 = constp.tile([1, TOWN], F32)
            nc.sync.dma_start(out=qpos1[:], in_=qpos_d[:])
            ones_row = constp.tile([1, P], F32)
            nc.vector.memset(ones_row[:], 1.0)
            oneone = constp.tile([1, 1], F32)
            nc.vector.memset(oneone[:], 1.0)
            qpos_bc = constp.tile([P, TOWN], F32)
            with tc.tile_pool(name="psb", bufs=2, space="PSUM") as psb:
                for w in range(NW):
                    bps = psb.tile([P, 512], F32, tag="bps", name="bps")
                    nc.tensor.matmul(
                        out=bps[:],
                        lhsT=ones_row[:],
                        rhs=qpos1[:, w * 512 : (w + 1) * 512],
                        start=True,
                        stop=True,
                    )
                    nc.vector.tensor_copy(
                        out=qpos_bc[:, w * 512 : (w + 1) * 512], in_=bps[:]
                    )
            idx_sb = constp.tile([P, NT], I32)
            nc.sync.dma_start(
                out=idx_sb[:], in_=idx_d[:].rearrange("(g r) -> r g", r=P)
            )

            attnT = {
                (i, w): persist.tile([P, 512], F32R, tag=f"attnT{i}_{w}", name=f"attnT{i}_{w}")
                for i in range(CCH)
                for w in range(NW)
            }
            acc8 = [persist.tile([P, NVC], F32, tag=f"acc{j}", name=f"acc{j}") for j in range(NQ)]

            # ---------- phases 1-3 under kvqp (K^T/V/Q^T span phases 2-3) ----------
            with tc.tile_pool(name="kvqp", bufs=1) as kvqp:
                kT = [kvqp.tile([P, T_], F32R, tag=f"kT{i}", name=f"kT{i}") for i in range(CCH)]
                vt = [kvqp.tile([P, C], F32R, tag=f"v{g}", name=f"v{g}") for g in range(NT)]
                qT = [kvqp.tile([P, TOWN], F32R, tag=f"qT{i}", name=f"qT{i}") for i in range(CCH)]
                with tc.tile_pool(name="xTp", bufs=1) as xTp:
                    xT = [xTp.tile([P, T_], F32R, tag=f"xT{i}", name=f"xT{i}") for i in range(CCH)]
                    # ----- phase 1: x^T = (gather token_emb)^T + pos^T -----
                    with (
                        tc.tile_pool(name="ph1", bufs=3) as ph1,
                        tc.tile_pool(name="posTp", bufs=1) as posTp,
                        tc.tile_pool(name="ps1", bufs=4, space="PSUM") as ps1,
                    ):
                        posT = []
                        for i in range(CCH):
                            pt = posTp.tile([P, T_], F32, tag=f"posT{i}", name=f"posT{i}")
                            nc.sync.dma_start(
                                out=pt[:], in_=post_d[i * P : (i + 1) * P, :]
                            )
                            posT.append(pt)
                        for g in range(NT):
                            xg = ph1.tile([P, C], F32, tag="xg", name="xg")
                            nc.gpsimd.indirect_dma_start(
                                out=xg[:],
                                out_offset=None,
                                in_=temb_d[:],
                                in_offset=bass.IndirectOffsetOnAxis(
                                    ap=idx_sb[:, g : g + 1], axis=0
                                ),
                            )
                            for i in range(CCH):
                                tp = ps1.tile([P, P], F32, tag="xtps", name="xtps")
                                nc.tensor.transpose(
                                    out=tp[:],
                                    in_=xg[:, i * P : (i + 1) * P],
                                    identity=ident[:],
                                )
                                nc.vector.tensor_tensor(
                                    out=xT[i][:, g * P : (g + 1) * P],
                                    in0=tp[:],
                                    in1=posT[i][:, g * P : (g + 1) * P],
                                    op=Alu.add,
                                )

                    # ----- phase 2: K^T [c,s], V [s,c], Q^T [c,q_own] -----
                    with (
                        tc.tile_pool(name="wqkv", bufs=1) as wqkv,
                        tc.tile_pool(name="ps2", bufs=2, space="PSUM") as ps2,
                    ):
                        wq_t = wqkv.tile([P, CCH, C], F32R)
                        wk_t = wqkv.tile([P, CCH, C], F32R)
                        wv_t = wqkv.tile([P, CCH, C], F32R)
                        for w_t, w_d in ((wq_t, wq_d), (wk_t, wk_d), (wv_t, wv_d)):
                            nc.sync.dma_start(
                                out=w_t[:], in_=w_d[:].rearrange("(k p) n -> p k n", p=P)
                            )
                        # emit per half (tok windows 0-1 then 2-3) so window-0
                        # attention can start while the second half is produced
                        for half in range(2):
                            w4s = range(half * (T_ // 1024), (half + 1) * (T_ // 1024))
                            for co in range(CCH):
                                for w4 in w4s:
                                    ps = ps2.tile([P, 512], F32, tag="kps", name="kps")
                                    for ci in range(CCH):
                                        nc.tensor.matmul(
                                            out=ps[:],
                                            lhsT=wk_t[:, ci, co * P : (co + 1) * P],
                                            rhs=xT[ci][:, w4 * 512 : (w4 + 1) * 512],
                                            start=(ci == 0),
                                            stop=(ci == CCH - 1),
                                        )
                                    nc.scalar.activation(
                                        out=kT[co][:, w4 * 512 : (w4 + 1) * 512],
                                        in_=ps[:],
                                        func=Copy,
                                    )
                            for g in range(half * (NT // 2), (half + 1) * (NT // 2)):
                                ps = ps2.tile([P, C], F32, tag="vps", name="vps")
                                for ci in range(CCH):
                                    nc.tensor.matmul(
                                        out=ps[:],
                                        lhsT=xT[ci][:, g * P : (g + 1) * P],
                                        rhs=wv_t[:, ci, :],
                                        start=(ci == 0),
                                        stop=(ci == CCH - 1),
                                    )
                                nc.scalar.activation(out=vt[g][:], in_=ps[:], func=Copy)
                            for co in range(CCH if half < NW else 0):
                                w = half
                                ps = ps2.tile([P, 512], F32, tag="qps", name="qps")
                                for ci in range(CCH):
                                    xv = xT[ci][:].rearrange("c (g r) -> c g r", r=P)
                                    rhs = xv[:, 2 * 4 * w : 2 * 4 * (w + 1) : 2, :]
                                    nc.tensor.matmul(
                                        out=ps[:],
                                        lhsT=wq_t[:, ci, co * P : (co + 1) * P],
                                        rhs=rhs,
                                        start=(ci == 0),
                                        stop=(ci == CCH - 1),
                                    )
                                nc.scalar.activation(
                                    out=qT[co][:, w * 512 : (w + 1) * 512],
                                    in_=ps[:],
                                    func=Copy,
                                    scale=scale,
                                )

                # ---------- phase 3: attention in S^T layout ----------
                se_w = [persist.tile([1, 512], F32, tag=f"se_w{w}", name=f"se_w{w}") for w in range(NW)]
                recip8 = persist.tile([P, NQ], F32, tag="recip8", name="recip8")
                with (
                        tc.tile_pool(name="pTp", bufs=1) as pTp,
                        tc.tile_pool(name="ph3", bufs=2) as ph3,
                        tc.tile_pool(name="ps3a", bufs=3, space="PSUM") as ps3a,
                        tc.tile_pool(name="ps3b", bufs=2, space="PSUM") as ps3b,
                        tc.tile_pool(name="ps3c", bufs=2, space="PSUM") as ps3c,
                    ):
                        for w in range(NW):
                            ns = 8 * (w + 1)
                            qsl = slice(w * 512, (w + 1) * 512)
                            se_ps = ps3c.tile([1, 512], F32, tag="se_ps", name="se_ps")
                            pT_w = []
                            for ls in range(ns):
                                sps = ps3b.tile([P, 512], F32, tag="s_ps", name="s_ps")
                                for ci in range(CCH):
                                    nc.tensor.matmul(
                                        out=sps[:],
                                        lhsT=kT[ci][:, ls * P : (ls + 1) * P],
                                        rhs=qT[ci][:, qsl],
                                        start=(ci == 0),
                                        stop=(ci == CCH - 1),
                                    )
                                pt = pTp.tile([P, 512], F32R, tag=f"pT{ls}", name=f"pT{ls}")
                                if ls >= 8 * w:
                                    mask = ph3.tile([P, 512], F32, tag="mask", name="mask")
                                    nc.vector.tensor_scalar(
                                        out=mask[:],
                                        in0=qpos_bc[:, qsl],
                                        scalar1=spos[:, ls : ls + 1],
                                        scalar2=MASK_NEG,
                                        op0=Alu.is_lt,
                                        op1=Alu.mult,
                                    )
                                    smask = ph3.tile([P, 512], F32, tag="smask", name="smask")
                                    nc.vector.tensor_tensor(
                                        out=smask[:], in0=sps[:], in1=mask[:], op=Alu.add
                                    )
                                    nc.scalar.activation(out=pt[:], in_=smask[:], func=Exp)
                                else:
                                    nc.scalar.activation(out=pt[:], in_=sps[:], func=Exp)
                                pT_w.append(pt)
                                nc.tensor.matmul(
                                    out=se_ps[:],
                                    lhsT=ones_r[:],
                                    rhs=pt[:],
                                    start=(ls == 0),
                                    stop=(ls == ns - 1),
                                )
                            nc.vector.tensor_copy(out=se_w[w][:], in_=se_ps[:])
                            # per-token 1/sumexp for this window's 4 token tiles
                            for jj in range(4):
                                j = 4 * w + jj
                                scp = ps3c.tile([P, 1], F32, tag="scp", name="scp", bufs=1)
                                nc.tensor.matmul(
                                    out=scp[:],
                                    lhsT=se_w[w][:, jj * P : (jj + 1) * P],
                                    rhs=oneone[:],
                                    start=True,
                                    stop=True,
                                )
                                nc.vector.reciprocal(out=recip8[:, j : j + 1], in_=scp[:])
                            for co in range(CCH):
                                aps = ps3a.tile([P, 512], F32, tag="attn_ps", name="attn_ps")
                                for ls in range(ns):
                                    nc.tensor.matmul(
                                        out=aps[:],
                                        lhsT=vt[ls][:, co * P : (co + 1) * P],
                                        rhs=pT_w[ls][:],
                                        start=(ls == 0),
                                        stop=(ls == ns - 1),
                                    )
                                # attn^T stays unnormalized; the softmax divide is
                                # folded into the lm_head eviction per token tile
                                nc.vector.tensor_copy(out=attnT[(co, w)][:], in_=aps[:])

            # ---------- phase 4: lm_head + CE ----------
            with (
                tc.tile_pool(name="ph4", bufs=3) as ph4,
                tc.tile_pool(name="wop", bufs=2) as wop,
                tc.tile_pool(name="ps4", bufs=6, space="PSUM") as ps4,
                tc.tile_pool(name="ps5", bufs=1, space="PSUM") as ps5,
                tc.tile_pool(name="ps6", bufs=1, space="PSUM") as ps6,
            ):
                # vocab groups of up to 4 x 512 chunks -> big contiguous DMAs
                vgroups = []
                k = 0
                while k < NVC:
                    grp = vch[k : k + 4]
                    vgroups.append(grp)
                    k += 4
                for grp in vgroups:
                    goff = grp[0][0]
                    gnv = sum(nv for _, nv in grp)
                    wo_g = []
                    for ci in range(CCH):
                        wg = wop.tile([P, 2048], F32R, tag=f"wo{ci}", name=f"wo{ci}", bufs=3)
                        nc.gpsimd.dma_start(
                            out=wg[:, :gnv],
                            in_=wo_d[ci * P : (ci + 1) * P, goff : goff + gnv],
                        )
                        wo_g.append(wg)
                    if with_bias:
                        bo_bc = ph4.tile([P, 2048], F32, tag="bo_bc", name="bo_bc", bufs=2)
                        nc.gpsimd.dma_start(
                            out=bo_bc[:, :gnv], in_=bo_d[:, goff : goff + gnv]
                        )
                    for j in range(NQ):
                        jw, jj = j // 4, j % 4
                        lg4 = ph4.tile([P, 2048], F32, tag="lg4", name="lg4", bufs=3)
                        for off, nv in grp:
                            loff = off - goff
                            vc = off // 512
                            ps = ps4.tile([P, nv], F32, tag="lm_ps", name="lm_ps")
                            for ci in range(CCH):
                                nc.tensor.matmul(
                                    out=ps[:],
                                    lhsT=attnT[(ci, jw)][:, jj * P : (jj + 1) * P],
                                    rhs=wo_g[ci][:, loff : loff + nv],
                                    start=(ci == 0),
                                    stop=(ci == CCH - 1),
                                )
                            nc.vector.tensor_scalar(
                                out=lg4[:, loff : loff + nv],
                                in0=ps[:],
                                scalar1=recip8[:, j : j + 1],
                                scalar2=None,
                                op0=Alu.mult,
                            )
                            if with_bias:
                                nc.vector.tensor_tensor(
                                    out=lg4[:, loff : loff + nv],
                                    in0=lg4[:, loff : loff + nv],
                                    in1=bo_bc[:, loff : loff + nv],
                                    op=Alu.add,
                                )
                                escr = ph4.tile([P, 512], F32, tag="escr", name="escr", bufs=2)
                                nc.scalar.activation(
                                    out=escr[:, :nv],
                                    in_=lg4[:, loff : loff + nv],
                                    func=Exp,
                                    accum_out=acc8[j][:, vc : vc + 1],
                                )
                            else:
                                escr = ph4.tile([P, 512], F32, tag="escr", name="escr", bufs=2)
                                nc.scalar.activation(
                                    out=escr[:, :nv],
                                    in_=ps[:],
                                    func=Exp,
                                    scale=recip8[:, j : j + 1],
                                    accum_out=acc8[j][:, vc : vc + 1],
                                )
                        nc.sync.dma_start(
                            out=logits_d[j * P : (j + 1) * P, goff : goff + gnv],
                            in_=lg4[:, :gnv],
                        )

                # ---- CE tail ----
                lse8 = ph4.tile([P, NQ], F32, tag="lse8", name="lse8", bufs=1)
                for j in range(NQ):
                    se = ph4.tile([P, 1], F32, tag="se_j", name="se_j", bufs=2)
                    nc.vector.reduce_sum(
                        out=se[:], in_=acc8[j][:], axis=mybir.AxisListType.X
                    )
                    nc.scalar.activation(out=lse8[:, j : j + 1], in_=se[:], func=Ln)
                # cross-partition sum of all NQ lse columns: one plain-f32 matmul
                loss_ps = ps5.tile([1, NQ], F32, tag="loss_ps", name="loss_ps")
                nc.tensor.matmul(
                    out=loss_ps[:],
                    lhsT=ones_r[:].bitcast(F32),
                    rhs=lse8[:],
                    start=True,
                    stop=True,
                )
                tgt_sb = ph4.tile([1, TOWN], F32, tag="tgt_sb", name="tgt_sb", bufs=1)
                for w in range(NW):
                    qsl = slice(w * 512, (w + 1) * 512)
                    tgt_ps = ps6.tile([1, 512], F32, tag="tgt_ps", name="tgt_ps")
                    for ci in range(CCH):
                        wtc = ph4.tile([P, 512], F32R, tag="wtc", name="wtc", bufs=2)
                        nc.gpsimd.dma_start(
                            out=wtc[:],
                            in_=wtgt_d[ci * P : (ci + 1) * P, qsl],
                        )
                        tg = ph4.tile([P, 512], F32R, tag="tg", name="tg", bufs=2)
                        nc.vector.tensor_tensor(
                            out=tg[:],
                            in0=attnT[(ci, w)][:],
                            in1=wtc[:],
                            op=Alu.mult,
                        )
                        nc.tensor.matmul(
                            out=tgt_ps[:],
                            lhsT=ones_r[:],
                            rhs=tg[:],
                            start=(ci == 0),
                            stop=(ci == CCH - 1),
                        )
                    nc.vector.tensor_copy(out=tgt_sb[:, qsl], in_=tgt_ps[:])
                rec_row = ph4.tile([1, TOWN], F32, tag="rec_row", name="rec_row", bufs=1)
                for w in range(NW):
                    nc.vector.reciprocal(
                        out=rec_row[:, w * 512 : (w + 1) * 512], in_=se_w[w][:]
                    )
                nc.vector.tensor_tensor(
                    out=tgt_sb[:], in0=tgt_sb[:], in1=rec_row[:], op=Alu.mult
                )
                botgt_sb = ph4.tile([1, TOWN], F32, tag="botgt", name="botgt", bufs=1)
                nc.gpsimd.dma_start(out=botgt_sb[:], in_=botgt_d[:])
                nc.vector.tensor_tensor(
                    out=tgt_sb[:], in0=tgt_sb[:], in1=botgt_sb[:], op=Alu.add
                )
                tgt_sum = ph4.tile([1, 1], F32, tag="tgt_sum", name="tgt_sum", bufs=1)
                nc.vector.reduce_sum(
                    out=tgt_sum[:], in_=tgt_sb[:], axis=mybir.AxisListType.X
                )
                lse_row = ph4.tile([1, NQ], F32, tag="lse_row", name="lse_row", bufs=1)
                nc.vector.tensor_copy(out=lse_row[:], in_=loss_ps[:])
                lse_sum = ph4.tile([1, 1], F32, tag="lse_sum", name="lse_sum", bufs=1)
                nc.vector.reduce_sum(
                    out=lse_sum[:], in_=lse_row[:], axis=mybir.AxisListType.X
                )
                loss_sb = ph4.tile([1, 1], F32, tag="loss_sb", name="loss_sb", bufs=1)
                nc.vector.tensor_tensor(
                    out=loss_sb[:], in0=lse_sum[:], in1=tgt_sum[:], op=Alu.subtract
                )
                nc.sync.dma_start(out=loss_d[:], in_=loss_sb[:])

    if do_compile:
        nc.compile()
    return nc


# ---------------------------------------------------------------------------
# host side
# ---------------------------------------------------------------------------

def _local_order(T_):
    """pos_of_local[p][t_local] = global position, per parity."""
    NT = T_ // P
    out = {}
    for p in (0, 1):
        gt = np.arange(NT)
        gtile = np.where(gt % 2 == 0, gt + p, gt - p)  # local tile -> global tile
        out[p] = (gtile[:, None] * P + np.arange(P)[None, :]).reshape(-1)
    return out


def make_in_maps(idx, targets, token_emb, pos_emb, Wk, Wq, Wv, Wo, bo, T_=T, V_=V):
    idx = np.asarray(idx).astype(np.int32)
    targets = np.asarray(targets).astype(np.int64)
    token_emb = np.ascontiguousarray(np.asarray(token_emb, np.float32))
    pos_emb_t = np.ascontiguousarray(np.asarray(pos_emb, np.float32).T)
    Wk = np.ascontiguousarray(np.asarray(Wk, np.float32))
    Wq = np.ascontiguousarray(np.asarray(Wq, np.float32))
    Wv = np.ascontiguousarray(np.asarray(Wv, np.float32))
    Wo = np.ascontiguousarray(np.asarray(Wo, np.float32))
    bo = np.asarray(bo, np.float32).reshape(1, V_)
    with_bias = bool(np.any(bo))

    NT = T_ // P
    TOWN = T_ // 2
    NQ = TOWN // P
    NVC = (V_ + 511) // 512
    order = _local_order(T_)
    spos_base = {
        p: (order[p].reshape(NT, P).T).astype(np.float32) for p in (0, 1)
    }  # [P, NT] global position of each local key slot
    ident = np.eye(P, dtype=np.float32)
    ones = np.ones((P, 1), np.float32)

    in_maps = []
    meta = []
    n_batches = idx.shape[0]
    for c in range(N_CORES):
        b, p = c // 2, c % 2
        pos_l = order[p]
        own_rows_local = np.concatenate(
            [np.arange(2 * j * P, (2 * j + 1) * P) for j in range(NQ)]
        )
        own_global = pos_l[own_rows_local]
        tgt_own = targets[b % n_batches][own_global]
        in_map = {
            "idx": idx[b % n_batches][pos_l],
            "token_emb": token_emb,
            "pos_emb_t": pos_emb_t[:, pos_l],
            "wq": Wq,
            "wk": Wk,
            "wv": Wv,
            "wo": Wo,
            "w_tgt": np.ascontiguousarray(Wo[:, tgt_own]),
            "bo_tgt": bo[:, tgt_own],
            "qpos": own_global[None, :].astype(np.float32),
            "spos": spos_base[p],
            "ident": ident,
            "ones_r": ones,
        }
        if with_bias:
            in_map["bo_bc"] = np.ascontiguousarray(np.broadcast_to(bo, (P, V_)))
        in_maps.append(in_map)
        meta.append((b % n_batches, own_global))
    return in_maps, meta, with_bias


_prog_cache = {}


def kernel(idx, targets, token_emb, pos_emb, Wk, Wq, Wv, Wo, bo):
    from concourse.bass_utils import run_bass_kernel_spmd

    in_maps, meta, with_bias = make_in_maps(
        idx, targets, token_emb, pos_emb, Wk, Wq, Wv, Wo, bo, T, V
    )
    key = (T, V, with_bias)
    if key not in _prog_cache:
        _prog_cache[key] = build_program(T, V, with_bias=with_bias)
    nc = _prog_cache[key]
    res = run_bass_kernel_spmd(nc, in_maps, list(range(N_CORES)))

    logits = np.empty((B, T, V), np.float32)
    loss_sum = 0.0
    for c in range(N_CORES):
        b, own_global = meta[c]
        logits[b][own_global] = res.results[c]["logits"]
        loss_sum += float(res.results[c]["loss_part"][0, 0])
    loss = np.float32(loss_sum / (B * T))
    return logits, loss
